# revision 17
# baseline (speedup 1.0000x reference)
"""Fused self-attention flow kernel for Trainium2 (8 NeuronCores).

Problem (hardcoded shapes): B=4, C=256, H=W=64, N=H*W=4096.
  x      = (inp [B,C,H,W] -> [B,N,C]) @ W_lin.T + b_lin
  scores = (x/16) @ x.T            # [B,N,N]
  attn   = softmax(scores, -1)
  out    = (attn @ flow [B,N,2]) -> [B,2,H,W]

Sharding: core c in 0..7 handles batch b=c//2, q-rows half=c%2.
Each core receives its batch's full inp (needed for K) *rolled* along N so
its own q-half occupies local rows 0..2047 -- every core runs the identical
SPMD program.

Per-core device program:
  1. xT[c_out, n] = W @ inp + b           (PE fp16 matmul + DVE bias-add)
  2. for each k-block (32 x 128):
       scoresT[k,q] (q=0..2047) in PSUM   (PE, accumulate over 2 C-chunks)
       probsT = exp(scoresT/16) -> SBUF   (ScalarE; no max-subtraction:
                                           scores <= ||x||^2/16 ~ 8 << 88)
       acc[3, q] += [f0,f1,1]^T @ probsT  (PE, 4-way column-tiled, PSUM acc)
  3. acc -> SBUF -> HBM "out" [12, 512]
     (col-group j holds q in [512j, 512j+512) on partitions 32j..32j+2)

Host: out[q,0:2] = acc[0:2,q]/acc[2,q], then unshard/reshape.
"""

import numpy as np

B, C, H, W = 4, 256, 64, 64
N = H * W          # 4096
QL = N // 2        # per-core q rows (2048)
KB = N // 128      # 32 k-blocks
NCORES = 8

_CACHE = {}


def _build_body(nc, tc, ctx, mybir, dram, fp8=False):
    f32 = mybir.dt.float32
    f16 = mybir.dt.float16
    inp_d, wt_d, bias_d, flow3_d, out_d = dram

    sb = ctx.enter_context(tc.tile_pool(name="sb", bufs=1))
    probs_pool = ctx.enter_context(tc.tile_pool(name="probs", bufs=4))
    lin_ps = ctx.enter_context(tc.tile_pool(name="lin_ps", bufs=2, space="PSUM"))
    sc_ps = ctx.enter_context(tc.tile_pool(name="sc_ps", bufs=2, space="PSUM"))
    out_ps_pool = ctx.enter_context(tc.tile_pool(name="out_ps", bufs=1, space="PSUM"))

    # --- constants / small inputs ---
    wt_sb = sb.tile([128, 2, C], f16)
    nc.sync.dma_start(out=wt_sb[:], in_=wt_d[:])
    bias_sb = sb.tile([128, 2], f32)
    nc.sync.dma_start(out=bias_sb[:], in_=bias_d[:])
    flow3_sb = sb.tile([128, KB, 3], f16)
    nc.sync.dma_start(out=flow3_sb[:], in_=flow3_d[:])

    # exp bias: exp(s/16 - 4) — cancels in the softmax ratio, keeps fp16
    # probs far from overflow
    exp_bias = sb.tile([128, 1], f32)
    nc.vector.memset(exp_bias[:], -4.0)

    # warm up the exp table-load (~2.7us) under the input DMA
    warm = sb.tile([128, 8], f32)
    nc.vector.memset(warm[:], 0.0)
    nc.scalar.activation(out=warm[:], in_=warm[:],
                         func=mybir.ActivationFunctionType.Exp)

    # --- inp DMA (n-tile major so the linear can start early; fp16 from host)
    # --- linear xT[oc*128+p, n] = sum_ic W.T[ic, oc] . inp[ic] + b
    f8 = mybir.dt.float8e4
    inp_f16 = [sb.tile([128, N], f16, name=f"inp_f16_{ic}", tag=f"inpf{ic}")
               for ic in range(2)]
    if fp8:
        xT8 = sb.tile([128, 2, N], f8, name="xT8", tag="xT8")
    else:
        xT = [sb.tile([128, N], f16, name=f"xT{oc}", tag=f"xT{oc}")
              for oc in range(2)]
    for nt in range(8):
        s = slice(nt * 512, (nt + 1) * 512)
        for ic in range(2):
            nc.sync.dma_start(out=inp_f16[ic][:, s],
                              in_=inp_d[ic * 128:(ic + 1) * 128, s])
        for oc in range(2):
            pl = lin_ps.tile([128, 512], f32)
            for ic in range(2):
                nc.tensor.matmul(
                    pl[:],
                    lhsT=wt_sb[:, ic, oc * 128:(oc + 1) * 128],
                    rhs=inp_f16[ic][:, s],
                    start=(ic == 0), stop=(ic == 1),
                )
            dst = xT8[:, oc, s] if fp8 else xT[oc][:, s]
            nc.vector.tensor_scalar_add(dst, pl[:], bias_sb[:, oc:oc + 1])

    # --- attention over local q rows 0..QL, all 4096 k ---
    out_acc = out_ps_pool.tile([128, 512], f32)
    for kb in range(KB):
        ks = slice(kb * 128, (kb + 1) * 128)
        pts = []
        for qt in range(2):  # two 1024-wide q sub-tiles
            ps = sc_ps.tile([128, 1024], f32, name="ps", tag="ps")
            for nn in range(2):
                qs = slice(qt * 1024 + nn * 512, qt * 1024 + (nn + 1) * 512)
                if fp8:
                    nc.tensor.matmul(
                        ps[:, nn * 512:(nn + 1) * 512],
                        lhsT=xT8[:, :, ks],
                        rhs=xT8[:, :, qs],
                        start=True, stop=True,
                        perf_mode=mybir.MatmulPerfMode.DoubleRow,
                    )
                else:
                    for ic in range(2):
                        nc.tensor.matmul(
                            ps[:, nn * 512:(nn + 1) * 512],
                            lhsT=xT[ic][:, ks],
                            rhs=xT[ic][:, qs],
                            start=(ic == 0), stop=(ic == 1),
                        )
            pt = probs_pool.tile([128, 1024], f16, name="pt", tag="pt")
            # exp(s/16 - 4): the constant shift cancels in the softmax ratio
            # (host divides num by den) and keeps fp16 probs far from overflow
            nc.scalar.activation(out=pt[:], in_=ps[:],
                                 func=mybir.ActivationFunctionType.Exp,
                                 scale=float(C) ** -0.5, bias=exp_bias[:])
            pts.append(pt)
        # 4 skinny accumulating matmuls back-to-back -> 4-way column-tiled
        # concurrency in the PE array
        for qt in range(2):
            for nn in range(2):
                j = qt * 2 + nn  # column-group / q-subtile 0..3
                nc.tensor.matmul(
                    out_acc[32 * j:32 * j + 3, :],
                    lhsT=flow3_sb[:, kb, :],
                    rhs=pts[qt][:, nn * 512:(nn + 1) * 512],
                    start=(kb == 0), stop=(kb == KB - 1),
                    tile_position=(0, 32 * j),
                    skip_group_check=True,
                )

    out_sb = sb.tile([128, 512], f32)
    for j in range(4):
        nc.vector.tensor_copy(out=out_sb[32 * j:32 * j + 3, :],
                              in_=out_acc[32 * j:32 * j + 3, :])
        nc.sync.dma_start(out=out_d[3 * j:3 * j + 3, :],
                          in_=out_sb[32 * j:32 * j + 3, :])


def _build_nc(reps=1, fp8=False):
    from contextlib import ExitStack

    import concourse.bacc as bacc
    import concourse.tile as tile
    from concourse import mybir

    f32 = mybir.dt.float32
    f16 = mybir.dt.float16

    nc = bacc.Bacc("TRN2", target_bir_lowering=False, debug=False)

    dram = (
        nc.dram_tensor("inp", (C, N), f16, kind="ExternalInput"),
        nc.dram_tensor("wt", (128, 2, C), f16, kind="ExternalInput"),
        nc.dram_tensor("bias", (128, 2), f32, kind="ExternalInput"),
        nc.dram_tensor("flow3", (128, KB, 3), f16, kind="ExternalInput"),
        nc.dram_tensor("out", (12, 512), f32, kind="ExternalOutput"),
    )

    with tile.TileContext(nc) as tc:
        for _ in range(reps):
            with ExitStack() as ctx:
                _build_body(nc, tc, ctx, mybir, dram, fp8=fp8)

    nc.compile()
    return nc


import os
_FP8 = os.environ.get("K_FP8", "0") == "1"


def _get_nc(reps=1):
    key = ("nc", reps, _FP8)
    if key not in _CACHE:
        _CACHE[key] = _build_nc(reps, fp8=_FP8)
    return _CACHE[key]


def _make_in_maps(inp, flow_init, W_lin, b_lin):
    inp = np.ascontiguousarray(np.asarray(inp, dtype=np.float32)).reshape(B, C, N)
    flow = np.ascontiguousarray(np.asarray(flow_init, dtype=np.float32)).reshape(B, 2, N)
    W_lin = np.asarray(W_lin, dtype=np.float32)
    b_lin = np.asarray(b_lin, dtype=np.float32)

    # lhsT layout for xT = W @ inp: [c_in(part 128), ic, c_out]
    wt = np.ascontiguousarray(
        W_lin.T.reshape(2, 128, C).transpose(1, 0, 2)).astype(np.float16)
    bias = np.ascontiguousarray(b_lin.reshape(2, 128).T)        # [128, 2]

    in_maps = []
    for c in range(NCORES):
        b, half = divmod(c, 2)
        sh = -QL * half
        inp_c = np.roll(inp[b], sh, axis=1) if half else inp[b]
        inp_c = inp_c.astype(np.float16)
        f = np.roll(flow[b], sh, axis=1) if half else flow[b]
        flow3 = np.empty((N, 3), np.float16)
        flow3[:, 0:2] = f.T
        flow3[:, 2] = 1.0
        flow3_c = np.ascontiguousarray(
            flow3.reshape(KB, 128, 3).transpose(1, 0, 2))       # [128, KB, 3]
        in_maps.append({
            "inp": np.ascontiguousarray(inp_c),
            "wt": wt,
            "bias": bias,
            "flow3": flow3_c,
        })
    return in_maps


def _postprocess(results):
    out = np.empty((B, 2, N), np.float32)
    for c in range(NCORES):
        b, half = divmod(c, 2)
        acc = results[c]["out"]                                  # [12, 512]
        groups = [acc[3 * j:3 * j + 3, :] for j in range(4)]
        a = np.concatenate(groups, axis=1)                       # [3, 2048]
        out[b, :, half * QL:(half + 1) * QL] = a[0:2] / a[2]
    return out.reshape(B, 2, H, W)


def _run(inputs, trace=False):
    from concourse.bass_utils import run_bass_kernel_spmd

    nc = _get_nc()
    in_maps = _make_in_maps(inputs["inp"], inputs["flow_init"],
                            inputs["W_lin"], inputs["b_lin"])
    r = run_bass_kernel_spmd(nc, in_maps, core_ids=list(range(NCORES)),
                             trace=False)
    _CACHE["last_exec_ns"] = r.exec_time_ns
    return _postprocess(r.results)


def kernel(**inputs) -> np.ndarray:
    return _run(inputs, trace=False)
